# revision 9
# baseline (speedup 1.0000x reference)
"""APPNP GNN kernel for 8 Trainium2 NeuronCores.

Sharding: nodes (dst side) split into 8 contiguous shards of 12500.
Per step: all-gather of the dinv-scaled bf16 feature table [N, 128]
(64 feats + 64 pad so gather rows are 256B), then each core gathers
per-edge source rows via dma_gather and segment-sums them by dst with
one-hot bf16 matmuls accumulating in PSUM.

Gather schedule is RANGE-major: edges sorted by (src-range, dst-block),
calls of up to 32 tiles (4096 idxs) span dst-blocks within a range to
amortize the per-call SWDGE descriptor-generation cost on the Pool
engine. Per-(range, block) PSUM partials are accumulated into an SBUF
block accumulator; blocks finalize after their last range group.

Norms fold into per-node scales: norm(s,d) = dinv[s]*dinv[d]; the
table is pre-scaled by dinv and the dst-side dinv applies at finalize
via the Activation engine's per-partition scale. Self-loops handled
analytically.
"""
import os
import sys

sys.path.insert(0, "/opt/trn_rl_repo")

import numpy as np
import ml_dtypes

BF16 = ml_dtypes.bfloat16

N = 100000
E = 3200000
IN = 256
HID = 256
OUT = 64
K = 3
ALPHA = 0.1
NCORES = 8
NLOC = N // NCORES          # 12500
NBLK = (NLOC + 127) // 128  # 98 dst blocks per core
RANGE = 32768               # int16 index range
NRANGES = (N + RANGE - 1) // RANGE  # 4
CT = 8                      # tiles per dma_gather call (1024 idxs)
PAD = 128                   # padded feature width (256B bf16 rows)


def _pack_idx_call(idx):
    """[n] int array -> [128, n//16] int16 tile (i -> [i%16, i//16], x8 replicated)."""
    n = len(idx)
    t = idx.reshape(n // 16, 16).T.astype(np.int16)
    return np.tile(t, (8, 1))


def _make_schedule(cnt_max):
    """cnt_max: [NRANGES, NBLK] max-over-cores bucket counts.

    Returns (groups, tiles, calls):
      groups: list of dicts {r, b, ntiles, first_for_block, last_for_block}
      tiles:  list of (group_idx, first_in_group, last_in_group)
      calls:  list of (r, ct) — tiles consumed in order
    """
    ntiles = np.ceil(cnt_max / 128.0).astype(np.int64)   # [NRANGES, NBLK]
    groups = []
    tiles = []
    first_seen = {}
    last_group_of_block = {}
    for r in range(NRANGES):
        for b in range(NBLK):
            t = int(ntiles[r, b])
            if t == 0:
                continue
            gi = len(groups)
            groups.append({"r": r, "b": b, "ntiles": t,
                           "first_for_block": b not in first_seen,
                           "last_for_block": False})
            first_seen[b] = True
            last_group_of_block[b] = gi
            for j in range(t):
                tiles.append((gi, j == 0, j == t - 1))
    for b, gi in last_group_of_block.items():
        groups[gi]["last_for_block"] = True
    # calls: chunk tiles into <=CT per call, never crossing a range boundary
    calls = []
    i = 0
    while i < len(tiles):
        r = groups[tiles[i][0]]["r"]
        ct = 0
        while i + ct < len(tiles) and ct < CT and groups[tiles[i + ct][0]]["r"] == r:
            ct += 1
        calls.append((r, ct))
        i += ct
    return groups, tiles, calls


def _preprocess(x, edge_index, W1, b1, W2, b2):
    src = np.asarray(edge_index[0], dtype=np.int64)
    dst = np.asarray(edge_index[1], dtype=np.int64)
    deg = np.bincount(dst, minlength=N).astype(np.float64) + 1.0  # +self loop
    dinv = (1.0 / np.sqrt(deg)).astype(np.float32)

    # per-core edge buckets, RANGE-major
    core_of = dst // NLOC
    per_core = []
    cnt = np.zeros((NCORES, NRANGES, NBLK), dtype=np.int64)
    for c in range(NCORES):
        m = core_of == c
        s_c = src[m]
        d_c = dst[m] - c * NLOC
        blk = d_c // 128
        drel = d_c % 128
        rng_ = s_c // RANGE
        order = np.lexsort((s_c, blk, rng_))   # rng major, then blk
        s_c, drel, blk, rng_ = s_c[order], drel[order], blk[order], rng_[order]
        np.add.at(cnt[c], (rng_, blk), 1)
        per_core.append((s_c, drel))

    cnt_max = cnt.max(axis=0)                      # [NRANGES, NBLK]
    groups, tiles, calls = _make_schedule(cnt_max)
    T_total = len(tiles)

    # per-core streams in the shared tile order
    idx_streams, dstrel_streams = [], []
    for c in range(NCORES):
        s_c, drel = per_core[c]
        starts = np.zeros(NRANGES * NBLK, dtype=np.int64)
        flat = cnt[c].reshape(-1)
        starts[1:] = np.cumsum(flat)[:-1]
        starts = starts.reshape(NRANGES, NBLK)
        cur = starts.copy()
        remaining = cnt[c].copy()
        idx_all = np.zeros(T_total * 128, dtype=np.int64)
        drel_all = np.full(T_total * 128, -1.0, dtype=np.float32)
        for ti, (gi, _, _) in enumerate(tiles):
            g = groups[gi]
            r, b = g["r"], g["b"]
            n_take = min(int(remaining[r, b]), 128)
            off = int(cur[r, b])
            idx_all[ti * 128:ti * 128 + n_take] = s_c[off:off + n_take] - r * RANGE
            drel_all[ti * 128:ti * 128 + n_take] = drel[off:off + n_take]
            cur[r, b] += n_take
            remaining[r, b] -= n_take
        # pack idx per call
        idx_cols = []
        tcur = 0
        for (r, ct) in calls:
            idx_cols.append(_pack_idx_call(idx_all[tcur * 128:(tcur + ct) * 128]))
            tcur += ct
        idx_streams.append(np.concatenate(idx_cols, axis=1))      # [128, 8*T_total]
        dstrel_streams.append(
            np.ascontiguousarray(drel_all.reshape(T_total, 128).T))  # [128, T_total]

    # per-core dense inputs
    in_maps = []
    for c in range(NCORES):
        lo, hi = c * NLOC, (c + 1) * NLOC
        xT = np.ascontiguousarray(x[lo:hi].T).astype(BF16)       # [256, 12500]
        dv = dinv[lo:hi]
        dinv_col = np.zeros((128, NBLK), np.float32)
        for b in range(NBLK):
            nblk = min(128, NLOC - b * 128)
            dinv_col[:nblk, b] = dv[b * 128:b * 128 + nblk]
        W1p = np.zeros((128, 512), np.float32)           # [k, (k*2+m) blocks]
        for k in range(2):
            for mblk in range(2):
                W1p[:, (k * 2 + mblk) * 128:(k * 2 + mblk + 1) * 128] = \
                    W1[k * 128:(k + 1) * 128, mblk * 128:(mblk + 1) * 128]
        W2p = np.zeros((128, 128), np.float32)           # [hid, m*64+o]
        for mblk in range(2):
            W2p[:, mblk * 64:(mblk + 1) * 64] = W2[mblk * 128:(mblk + 1) * 128, :]
        b1c = np.stack([b1[:128], b1[128:]], axis=1).astype(np.float32)   # [128, 2]
        b2r = np.tile(b2[None, :], (128, 1)).astype(np.float32)           # [128, 64]
        in_maps.append({
            "xT": xT,
            "idxs": idx_streams[c],
            "dstrel": dstrel_streams[c],
            "dinvc": dinv_col,
            "W1p": W1p.astype(BF16), "W2p": W2p.astype(BF16),
            "b1c": b1c, "b2r": b2r,
        })
    return in_maps, groups, tiles, calls, T_total


def _build(groups, tiles, calls, T_total):
    from concourse import bass, bacc, mybir
    from concourse.tile import TileContext

    DT = mybir.dt.float32
    BF = mybir.dt.bfloat16
    nc = bacc.Bacc("TRN2", target_bir_lowering=False, debug=False,
                   num_devices=NCORES, num_swdge_queues=4)

    xT_d = nc.dram_tensor("xT", [IN, NLOC], BF, kind="ExternalInput").ap()
    idxs_d = nc.dram_tensor("idxs", [128, 8 * T_total], mybir.dt.int16,
                            kind="ExternalInput").ap()
    dstrel_d = nc.dram_tensor("dstrel", [128, T_total], DT,
                              kind="ExternalInput").ap()
    dinvc_d = nc.dram_tensor("dinvc", [128, NBLK], DT, kind="ExternalInput").ap()
    W1p_d = nc.dram_tensor("W1p", [128, 512], BF, kind="ExternalInput").ap()
    W2p_d = nc.dram_tensor("W2p", [128, 128], BF, kind="ExternalInput").ap()
    b1c_d = nc.dram_tensor("b1c", [128, 2], DT, kind="ExternalInput").ap()
    b2r_d = nc.dram_tensor("b2r", [128, 64], DT, kind="ExternalInput").ap()
    out_d = nc.dram_tensor("out", [NLOC, OUT], DT, kind="ExternalOutput").ap()

    outp_loc = nc.dram_tensor("outp_loc", [NLOC, PAD], BF).ap()
    tables = [nc.dram_tensor(f"table{k}", [N, PAD], BF, addr_space="Shared").ap()
              for k in range(K)]

    Copy = mybir.ActivationFunctionType.Copy
    Relu = mybir.ActivationFunctionType.Relu
    AD = mybir.AluOpType.add
    MU = mybir.AluOpType.mult
    EQ = mybir.AluOpType.is_equal

    with TileContext(nc) as tc:
        with tc.tile_pool(name="const", bufs=1) as constp, \
             tc.tile_pool(name="persist", bufs=1) as persist:
            iota_bf = constp.tile([128, CT * 128], BF)
            iota_f = constp.tile([128, CT * 128], DT)
            iota_i = constp.tile([128, CT * 128], mybir.dt.int32)
            nc.gpsimd.iota(iota_i[:], pattern=[[0, CT], [1, 128]],
                           base=0, channel_multiplier=0)
            nc.vector.tensor_copy(iota_bf[:], iota_i[:])
            nc.vector.tensor_copy(iota_f[:], iota_i[:])
            dinv_sb = constp.tile([128, NBLK], DT)
            nc.sync.dma_start(out=dinv_sb[:], in_=dinvc_d[:])
            A_sb = constp.tile([128, NBLK], DT)      # 0.9*dinv^2
            nc.vector.tensor_tensor(out=A_sb[:], in0=dinv_sb[:], in1=dinv_sb[:],
                                    op=MU)
            nc.vector.tensor_scalar_mul(A_sb[:], A_sb[:], 1.0 - ALPHA)
            C_sb = constp.tile([128, NBLK], DT)      # 0.9*dinv
            nc.vector.tensor_scalar_mul(C_sb[:], dinv_sb[:], 1.0 - ALPHA)

            dstrel_f = persist.tile([128, T_total], DT)
            dstrel_bf = persist.tile([128, T_total], BF)
            B_sb = persist.tile([128, NBLK * OUT], DT)   # 0.1*dinv*h
            D_sb = persist.tile([128, NBLK * OUT], DT)   # 0.1*h
            outp_sb = persist.tile([128, NBLK * OUT], DT)  # dinv*out_prev
            agg_sb = persist.tile([128, NBLK * OUT], DT)   # gather partials
            outp_pad = persist.tile([128, NBLK * PAD], BF)
            nc.gpsimd.memset(outp_pad[:], 0.0)

            nc.sync.dma_start(out=dstrel_f[:], in_=dstrel_d[:])
            nc.vector.tensor_copy(dstrel_bf[:], dstrel_f[:])

            # ---------------- MLP ----------------
            with tc.tile_pool(name="mlpw", bufs=1) as mlpw, \
                 tc.tile_pool(name="mlp", bufs=3) as mlp, \
                 tc.tile_pool(name="mpsum", bufs=2, space="PSUM") as mpsum, \
                 tc.tile_pool(name="mpsum2", bufs=2, space="PSUM") as mpsum2:
                W1_sb = mlpw.tile([128, 512], BF)
                nc.sync.dma_start(out=W1_sb[:], in_=W1p_d[:])
                W2_sb = mlpw.tile([128, 128], BF)
                nc.sync.dma_start(out=W2_sb[:], in_=W2p_d[:])
                b1_sb = mlpw.tile([128, 2], DT)
                nc.sync.dma_start(out=b1_sb[:], in_=b1c_d[:])
                b2_sb = mlpw.tile([128, 64], DT)
                nc.sync.dma_start(out=b2_sb[:], in_=b2r_d[:])

                for rb in range(NBLK):
                    r0 = rb * 128
                    nrow = min(128, NLOC - r0)
                    xt = mlp.tile([128, 2 * 128], BF, tag="xt")
                    for k in range(2):
                        nc.sync.dma_start(
                            out=xt[:, k * 128:k * 128 + nrow],
                            in_=xT_d[k * 128:(k + 1) * 128, r0:r0 + nrow])
                    h1 = mlp.tile([128, 2 * 128], BF, tag="h1")
                    for mblk in range(2):
                        p1 = mpsum.tile([128, 128], DT, tag="p1")
                        for k in range(2):
                            nc.tensor.matmul(
                                out=p1[:, :nrow],
                                lhsT=W1_sb[:, (k * 2 + mblk) * 128:(k * 2 + mblk + 1) * 128],
                                rhs=xt[:, k * 128:k * 128 + nrow],
                                start=(k == 0), stop=(k == 1))
                        nc.scalar.activation(
                            h1[:, mblk * 128:mblk * 128 + nrow], p1[:, :nrow],
                            Relu, bias=b1_sb[:, mblk:mblk + 1])
                    p2 = mpsum2.tile([128, 64], DT, tag="p2")
                    for mblk in range(2):
                        nc.tensor.matmul(
                            out=p2[:nrow, :],
                            lhsT=h1[:, mblk * 128:mblk * 128 + nrow],
                            rhs=W2_sb[:, mblk * 64:(mblk + 1) * 64],
                            start=(mblk == 0), stop=(mblk == 1))
                    ht = mlp.tile([128, 64], DT, tag="ht")
                    nc.vector.tensor_tensor(out=ht[:nrow], in0=p2[:nrow],
                                            in1=b2_sb[:nrow], op=AD)
                    ob = slice(rb * OUT, rb * OUT + OUT)
                    pb = slice(rb * PAD, rb * PAD + OUT)
                    # outp = dinv*h (Act, per-partition scale); D = 0.1*h; B = 0.1*outp
                    nc.scalar.activation(outp_sb[:nrow, ob], ht[:nrow], Copy,
                                         scale=dinv_sb[:nrow, rb:rb + 1])
                    nc.scalar.activation(D_sb[:nrow, ob], ht[:nrow], Copy,
                                         scale=ALPHA)
                    nc.scalar.activation(B_sb[:nrow, ob], outp_sb[:nrow, ob], Copy,
                                         scale=ALPHA)
                    nc.vector.tensor_copy(outp_pad[:nrow, pb], outp_sb[:nrow, ob])
                    nc.sync.dma_start(
                        out=outp_loc[r0:r0 + nrow, :],
                        in_=outp_pad[:nrow, rb * PAD:(rb + 1) * PAD])

            # ---------------- propagation ----------------
            range_lens = [min(RANGE, N - r * RANGE) for r in range(NRANGES)]
            # per-step experiment config: (single_packet, mask_mode)
            step_cfg = [(True, "f32bf"), (False, "bfbf"), (True, "f32f32")]
            for step in range(K):
                sp_mode, mask_mode = step_cfg[step]
                table = tables[step]
                nc.gpsimd.collective_compute(
                    "AllGather", mybir.AluOpType.bypass,
                    replica_groups=[list(range(NCORES))],
                    ins=[outp_loc[:].opt()],
                    outs=[table[:].opt()])
                with tc.tile_pool(name=f"gat{step}", bufs=3) as gat, \
                     tc.tile_pool(name=f"idx{step}", bufs=3) as idxp, \
                     tc.tile_pool(name=f"sbl{step}", bufs=3) as sbl, \
                     tc.tile_pool(name=f"sb32{step}", bufs=3) as sbl32, \
                     tc.tile_pool(name=f"ev{step}", bufs=4) as evp, \
                     tc.tile_pool(name=f"ps{step}", bufs=1, space="PSUM") as psp:
                    group_ps = {}
                    tile_cursor = 0
                    for ci, (r, ct) in enumerate(calls):
                        it = idxp.tile([128, CT * 8], mybir.dt.int16, tag="it")
                        nc.sync.dma_start(
                            out=it[:, :ct * 8],
                            in_=idxs_d[:, tile_cursor * 8:(tile_cursor + ct) * 8])
                        gt = gat.tile([128, CT, PAD], BF, tag="gt")
                        nc.gpsimd.dma_gather(
                            gt[:, :ct, :],
                            table[r * RANGE:r * RANGE + range_lens[r]],
                            it[:, :ct * 8],
                            ct * 128, ct * 128, PAD,
                            single_packet=sp_mode,
                            queue_num=ci % 4)
                        st = sbl.tile([128, CT * 128], BF, tag="st")
                        if mask_mode == "bfbf":
                            nc.vector.tensor_tensor(
                                out=st[:, :ct * 128].rearrange(
                                    "p (a b) -> p a b", b=128),
                                in0=dstrel_bf[:, tile_cursor:tile_cursor + ct]
                                    .unsqueeze(2).to_broadcast([128, ct, 128]),
                                in1=iota_bf[:, :ct * 128].rearrange(
                                    "p (a b) -> p a b", b=128),
                                op=EQ)
                        elif mask_mode == "f32bf":
                            nc.vector.tensor_tensor(
                                out=st[:, :ct * 128].rearrange(
                                    "p (a b) -> p a b", b=128),
                                in0=dstrel_f[:, tile_cursor:tile_cursor + ct]
                                    .unsqueeze(2).to_broadcast([128, ct, 128]),
                                in1=iota_f[:, :ct * 128].rearrange(
                                    "p (a b) -> p a b", b=128),
                                op=EQ)
                        else:  # f32f32 + cast
                            st32 = sbl32.tile([128, CT * 128], DT, tag="st32")
                            nc.vector.tensor_tensor(
                                out=st32[:, :ct * 128].rearrange(
                                    "p (a b) -> p a b", b=128),
                                in0=dstrel_f[:, tile_cursor:tile_cursor + ct]
                                    .unsqueeze(2).to_broadcast([128, ct, 128]),
                                in1=iota_f[:, :ct * 128].rearrange(
                                    "p (a b) -> p a b", b=128),
                                op=EQ)
                            nc.vector.tensor_copy(st[:, :ct * 128],
                                                  st32[:, :ct * 128])
                        done_groups = []
                        for j in range(ct):
                            gi, first, last = tiles[tile_cursor + j]
                            if first:
                                group_ps[gi] = psp.tile(
                                    [128, OUT], DT, tag=f"g{gi % 4}",
                                    name=f"ps{step}_{gi % 4}")
                            nc.tensor.matmul(
                                out=group_ps[gi][:, :],
                                lhsT=st[:, j * 128:(j + 1) * 128],
                                rhs=gt[:, j, 0:OUT],
                                start=first, stop=last)
                            if last:
                                done_groups.append(gi)
                        tile_cursor += ct
                        for gi in done_groups:
                            g = groups[gi]
                            b = g["b"]
                            nd = min(128, NLOC - b * 128)
                            ob = slice(b * OUT, b * OUT + OUT)
                            ps = group_ps.pop(gi)
                            if g["first_for_block"]:
                                nc.vector.tensor_copy(agg_sb[:, ob], ps[:, :])
                            else:
                                nc.vector.tensor_tensor(
                                    out=agg_sb[:, ob], in0=agg_sb[:, ob],
                                    in1=ps[:, :], op=AD)
                            if not g["last_for_block"]:
                                continue
                            # finalize block b
                            nc.vector.tensor_tensor(
                                out=agg_sb[:nd, ob], in0=agg_sb[:nd, ob],
                                in1=outp_sb[:nd, ob], op=AD)
                            if step < K - 1:
                                nc.scalar.activation(
                                    outp_sb[:nd, ob], agg_sb[:nd, ob], Copy,
                                    scale=A_sb[:nd, b:b + 1])
                                nc.vector.tensor_tensor(
                                    out=outp_sb[:nd, ob], in0=outp_sb[:nd, ob],
                                    in1=B_sb[:nd, ob], op=AD)
                                pb = slice(b * PAD, b * PAD + OUT)
                                nc.vector.tensor_copy(outp_pad[:nd, pb],
                                                      outp_sb[:nd, ob])
                                nc.sync.dma_start(
                                    out=outp_loc[b * 128:b * 128 + nd, :],
                                    in_=outp_pad[:nd, b * PAD:(b + 1) * PAD])
                            else:
                                res = evp.tile([128, OUT], DT, tag="res")
                                nc.scalar.activation(
                                    res[:nd], agg_sb[:nd, ob], Copy,
                                    scale=C_sb[:nd, b:b + 1])
                                nc.vector.tensor_tensor(
                                    out=res[:nd], in0=res[:nd],
                                    in1=D_sb[:nd, ob], op=AD)
                                nc.sync.dma_start(
                                    out=out_d[b * 128:b * 128 + nd, :],
                                    in_=res[:nd])
    nc.finalize()
    return nc


def kernel(x, edge_index, W1, b1, W2, b2):
    from concourse.bass_utils import run_bass_kernel_spmd

    in_maps, groups, tiles, calls, T_total = _preprocess(
        x, edge_index, W1, b1, W2, b2)
    nc = _build(groups, tiles, calls, T_total)
    trace = bool(int(os.environ.get("KERNEL_TRACE", "0")))
    if trace:
        import types
        mod = types.ModuleType("antenv.axon_hooks")
        mod._HOOK = None
        def _s(h): mod._HOOK = h
        def _g(): return mod._HOOK
        mod.set_axon_ntff_profile_hook = _s
        mod.get_axon_ntff_profile_hook = _g
        sys.modules["antenv.axon_hooks"] = mod
        import antenv
        antenv.axon_hooks = mod
        from trn_agent_boot.trn_boot import _ntff_profile_via_ctypes
        _s(_ntff_profile_via_ctypes('/opt/axon/libaxon_pjrt.so'))
        import concourse.bass_utils as bu
        bu.upload_artifacts = lambda tmpdir: "local://" + tmpdir
    res = run_bass_kernel_spmd(nc, in_maps, list(range(NCORES)), trace=trace)
    if trace and res.exec_time_ns:
        print(f"HW exec time: {res.exec_time_ns} ns")
    out = np.concatenate([res.results[c]["out"] for c in range(NCORES)], axis=0)
    return out


# revision 11
# speedup vs baseline: 1.5573x; 1.5573x over previous
"""APPNP GNN kernel for 8 Trainium2 NeuronCores.

Sharding: nodes (dst side) split into 8 contiguous shards of 12500.
Per step: all-gather of the dinv-scaled bf16 feature table [N, 128]
(64 feats + 64 pad so gather rows are 256B), then each core gathers
per-edge source rows via dma_gather and segment-sums them by dst with
one-hot bf16 matmuls accumulating in PSUM.

Gather schedule is RANGE-major: edges sorted by (src-range, dst-block),
calls of up to 32 tiles (4096 idxs) span dst-blocks within a range to
amortize the per-call SWDGE descriptor-generation cost on the Pool
engine. Per-(range, block) PSUM partials are accumulated into an SBUF
block accumulator; blocks finalize after their last range group.

Norms fold into per-node scales: norm(s,d) = dinv[s]*dinv[d]; the
table is pre-scaled by dinv and the dst-side dinv applies at finalize
via the Activation engine's per-partition scale. Self-loops handled
analytically.
"""
import os
import sys

sys.path.insert(0, "/opt/trn_rl_repo")

import numpy as np
import ml_dtypes

BF16 = ml_dtypes.bfloat16

N = 100000
E = 3200000
IN = 256
HID = 256
OUT = 64
K = 3
ALPHA = 0.1
NCORES = 8
NLOC = N // NCORES          # 12500
NBLK = (NLOC + 127) // 128  # 98 dst blocks per core
RANGE = 32768               # int16 index range
NRANGES = (N + RANGE - 1) // RANGE  # 4
CT = 16                     # tiles per dma_gather call (2048 idxs)
PAD = 128                   # padded feature width (256B bf16 rows)


def _pack_idx_call(idx):
    """[n] int array -> [128, n//16] int16 tile (i -> [i%16, i//16], x8 replicated)."""
    n = len(idx)
    t = idx.reshape(n // 16, 16).T.astype(np.int16)
    return np.tile(t, (8, 1))


def _make_schedule(cnt_max):
    """cnt_max: [NRANGES, NBLK] max-over-cores bucket counts.

    Returns (groups, tiles, calls):
      groups: list of dicts {r, b, ntiles, first_for_block, last_for_block}
      tiles:  list of (group_idx, first_in_group, last_in_group)
      calls:  list of (r, ct) — tiles consumed in order
    """
    ntiles = np.ceil(cnt_max / 128.0).astype(np.int64)   # [NRANGES, NBLK]
    groups = []
    tiles = []
    first_seen = {}
    last_group_of_block = {}
    for r in range(NRANGES):
        for b in range(NBLK):
            t = int(ntiles[r, b])
            if t == 0:
                continue
            gi = len(groups)
            groups.append({"r": r, "b": b, "ntiles": t,
                           "first_for_block": b not in first_seen,
                           "last_for_block": False})
            first_seen[b] = True
            last_group_of_block[b] = gi
            for j in range(t):
                tiles.append((gi, j == 0, j == t - 1))
    for b, gi in last_group_of_block.items():
        groups[gi]["last_for_block"] = True
    # calls: chunk tiles into <=CT per call, never crossing a range boundary
    calls = []
    i = 0
    while i < len(tiles):
        r = groups[tiles[i][0]]["r"]
        ct = 0
        while i + ct < len(tiles) and ct < CT and groups[tiles[i + ct][0]]["r"] == r:
            ct += 1
        calls.append((r, ct))
        i += ct
    return groups, tiles, calls


def _preprocess(x, edge_index, W1, b1, W2, b2):
    src = np.asarray(edge_index[0], dtype=np.int64)
    dst = np.asarray(edge_index[1], dtype=np.int64)
    deg = np.bincount(dst, minlength=N).astype(np.float64) + 1.0  # +self loop
    dinv = (1.0 / np.sqrt(deg)).astype(np.float32)

    # per-core edge buckets, RANGE-major
    core_of = dst // NLOC
    per_core = []
    cnt = np.zeros((NCORES, NRANGES, NBLK), dtype=np.int64)
    for c in range(NCORES):
        m = core_of == c
        s_c = src[m]
        d_c = dst[m] - c * NLOC
        blk = d_c // 128
        drel = d_c % 128
        rng_ = s_c // RANGE
        order = np.lexsort((s_c, blk, rng_))   # rng major, then blk
        s_c, drel, blk, rng_ = s_c[order], drel[order], blk[order], rng_[order]
        np.add.at(cnt[c], (rng_, blk), 1)
        per_core.append((s_c, drel))

    cnt_max = cnt.max(axis=0)                      # [NRANGES, NBLK]
    groups, tiles, calls = _make_schedule(cnt_max)
    T_total = len(tiles)

    # per-core streams in the shared tile order
    idx_streams, dstrel_streams = [], []
    for c in range(NCORES):
        s_c, drel = per_core[c]
        starts = np.zeros(NRANGES * NBLK, dtype=np.int64)
        flat = cnt[c].reshape(-1)
        starts[1:] = np.cumsum(flat)[:-1]
        starts = starts.reshape(NRANGES, NBLK)
        cur = starts.copy()
        remaining = cnt[c].copy()
        idx_all = np.zeros(T_total * 128, dtype=np.int64)
        drel_all = np.full(T_total * 128, -1.0, dtype=np.float32)
        for ti, (gi, _, _) in enumerate(tiles):
            g = groups[gi]
            r, b = g["r"], g["b"]
            n_take = min(int(remaining[r, b]), 128)
            off = int(cur[r, b])
            idx_all[ti * 128:ti * 128 + n_take] = s_c[off:off + n_take] - r * RANGE
            drel_all[ti * 128:ti * 128 + n_take] = drel[off:off + n_take]
            cur[r, b] += n_take
            remaining[r, b] -= n_take
        # pack idx per call
        idx_cols = []
        tcur = 0
        for (r, ct) in calls:
            idx_cols.append(_pack_idx_call(idx_all[tcur * 128:(tcur + ct) * 128]))
            tcur += ct
        idx_streams.append(np.concatenate(idx_cols, axis=1))      # [128, 8*T_total]
        dstrel_streams.append(
            np.ascontiguousarray(drel_all.reshape(T_total, 128).T))  # [128, T_total]

    # per-core dense inputs
    in_maps = []
    for c in range(NCORES):
        lo, hi = c * NLOC, (c + 1) * NLOC
        xT = np.ascontiguousarray(x[lo:hi].T).astype(BF16)       # [256, 12500]
        dv = dinv[lo:hi]
        dinv_col = np.zeros((128, NBLK), np.float32)
        for b in range(NBLK):
            nblk = min(128, NLOC - b * 128)
            dinv_col[:nblk, b] = dv[b * 128:b * 128 + nblk]
        W1p = np.zeros((128, 512), np.float32)           # [k, (k*2+m) blocks]
        for k in range(2):
            for mblk in range(2):
                W1p[:, (k * 2 + mblk) * 128:(k * 2 + mblk + 1) * 128] = \
                    W1[k * 128:(k + 1) * 128, mblk * 128:(mblk + 1) * 128]
        W2p = np.zeros((128, 128), np.float32)           # [hid, m*64+o]
        for mblk in range(2):
            W2p[:, mblk * 64:(mblk + 1) * 64] = W2[mblk * 128:(mblk + 1) * 128, :]
        b1c = np.stack([b1[:128], b1[128:]], axis=1).astype(np.float32)   # [128, 2]
        b2r = np.tile(b2[None, :], (128, 1)).astype(np.float32)           # [128, 64]
        in_maps.append({
            "xT": xT,
            "idxs": idx_streams[c],
            "dstrel": dstrel_streams[c],
            "dinvc": dinv_col,
            "W1p": W1p.astype(BF16), "W2p": W2p.astype(BF16),
            "b1c": b1c, "b2r": b2r,
        })
    return in_maps, groups, tiles, calls, T_total


def _build(groups, tiles, calls, T_total):
    from concourse import bass, bacc, mybir
    from concourse.tile import TileContext

    DT = mybir.dt.float32
    BF = mybir.dt.bfloat16
    nc = bacc.Bacc("TRN2", target_bir_lowering=False, debug=False,
                   num_devices=NCORES, num_swdge_queues=4)

    xT_d = nc.dram_tensor("xT", [IN, NLOC], BF, kind="ExternalInput").ap()
    idxs_d = nc.dram_tensor("idxs", [128, 8 * T_total], mybir.dt.int16,
                            kind="ExternalInput").ap()
    dstrel_d = nc.dram_tensor("dstrel", [128, T_total], DT,
                              kind="ExternalInput").ap()
    dinvc_d = nc.dram_tensor("dinvc", [128, NBLK], DT, kind="ExternalInput").ap()
    W1p_d = nc.dram_tensor("W1p", [128, 512], BF, kind="ExternalInput").ap()
    W2p_d = nc.dram_tensor("W2p", [128, 128], BF, kind="ExternalInput").ap()
    b1c_d = nc.dram_tensor("b1c", [128, 2], DT, kind="ExternalInput").ap()
    b2r_d = nc.dram_tensor("b2r", [128, 64], DT, kind="ExternalInput").ap()
    out_d = nc.dram_tensor("out", [NLOC, OUT], DT, kind="ExternalOutput").ap()

    outp_loc = nc.dram_tensor("outp_loc", [NLOC, PAD], BF).ap()
    tables = [nc.dram_tensor(f"table{k}", [N, PAD], BF, addr_space="Shared").ap()
              for k in range(K)]

    Copy = mybir.ActivationFunctionType.Copy
    Relu = mybir.ActivationFunctionType.Relu
    AD = mybir.AluOpType.add
    MU = mybir.AluOpType.mult
    EQ = mybir.AluOpType.is_equal

    with TileContext(nc) as tc:
        with tc.tile_pool(name="const", bufs=1) as constp, \
             tc.tile_pool(name="persist", bufs=1) as persist:
            iota_bf = constp.tile([128, CT * 128], BF)
            iota_i = constp.tile([128, CT * 128], mybir.dt.int32)
            nc.gpsimd.iota(iota_i[:], pattern=[[0, CT], [1, 128]],
                           base=0, channel_multiplier=0)
            nc.vector.tensor_copy(iota_bf[:], iota_i[:])
            dinv_sb = constp.tile([128, NBLK], DT)
            nc.sync.dma_start(out=dinv_sb[:], in_=dinvc_d[:])
            A_sb = constp.tile([128, NBLK], DT)      # 0.9*dinv^2
            nc.vector.tensor_tensor(out=A_sb[:], in0=dinv_sb[:], in1=dinv_sb[:],
                                    op=MU)
            nc.vector.tensor_scalar_mul(A_sb[:], A_sb[:], 1.0 - ALPHA)
            C_sb = constp.tile([128, NBLK], DT)      # 0.9*dinv
            nc.vector.tensor_scalar_mul(C_sb[:], dinv_sb[:], 1.0 - ALPHA)

            dstrel_bf = persist.tile([128, T_total], BF)
            B_sb = persist.tile([128, NBLK * OUT], DT)   # 0.1*dinv*h
            D_sb = persist.tile([128, NBLK * OUT], DT)   # 0.1*h
            outp_sb = persist.tile([128, NBLK * OUT], DT)  # dinv*out_prev
            agg_sb = persist.tile([128, NBLK * OUT], DT)   # gather partials
            outp_pad = persist.tile([128, NBLK * PAD], BF)
            nc.gpsimd.memset(outp_pad[:], 0.0)

            with tc.tile_pool(name="stg", bufs=1) as stgp:
                dstrel_f = stgp.tile([128, T_total], DT)
                nc.sync.dma_start(out=dstrel_f[:], in_=dstrel_d[:])
                nc.vector.tensor_copy(dstrel_bf[:], dstrel_f[:])

            # ---------------- MLP ----------------
            with tc.tile_pool(name="mlpw", bufs=1) as mlpw, \
                 tc.tile_pool(name="mlp", bufs=3) as mlp, \
                 tc.tile_pool(name="mpsum", bufs=2, space="PSUM") as mpsum, \
                 tc.tile_pool(name="mpsum2", bufs=2, space="PSUM") as mpsum2:
                W1_sb = mlpw.tile([128, 512], BF)
                nc.sync.dma_start(out=W1_sb[:], in_=W1p_d[:])
                W2_sb = mlpw.tile([128, 128], BF)
                nc.sync.dma_start(out=W2_sb[:], in_=W2p_d[:])
                b1_sb = mlpw.tile([128, 2], DT)
                nc.sync.dma_start(out=b1_sb[:], in_=b1c_d[:])
                b2_sb = mlpw.tile([128, 64], DT)
                nc.sync.dma_start(out=b2_sb[:], in_=b2r_d[:])

                for rb in range(NBLK):
                    r0 = rb * 128
                    nrow = min(128, NLOC - r0)
                    xt = mlp.tile([128, 2 * 128], BF, tag="xt")
                    for k in range(2):
                        nc.sync.dma_start(
                            out=xt[:, k * 128:k * 128 + nrow],
                            in_=xT_d[k * 128:(k + 1) * 128, r0:r0 + nrow])
                    h1 = mlp.tile([128, 2 * 128], BF, tag="h1")
                    for mblk in range(2):
                        p1 = mpsum.tile([128, 128], DT, tag="p1")
                        for k in range(2):
                            nc.tensor.matmul(
                                out=p1[:, :nrow],
                                lhsT=W1_sb[:, (k * 2 + mblk) * 128:(k * 2 + mblk + 1) * 128],
                                rhs=xt[:, k * 128:k * 128 + nrow],
                                start=(k == 0), stop=(k == 1))
                        nc.scalar.activation(
                            h1[:, mblk * 128:mblk * 128 + nrow], p1[:, :nrow],
                            Relu, bias=b1_sb[:, mblk:mblk + 1])
                    p2 = mpsum2.tile([128, 64], DT, tag="p2")
                    for mblk in range(2):
                        nc.tensor.matmul(
                            out=p2[:nrow, :],
                            lhsT=h1[:, mblk * 128:mblk * 128 + nrow],
                            rhs=W2_sb[:, mblk * 64:(mblk + 1) * 64],
                            start=(mblk == 0), stop=(mblk == 1))
                    ht = mlp.tile([128, 64], DT, tag="ht")
                    nc.vector.tensor_tensor(out=ht[:nrow], in0=p2[:nrow],
                                            in1=b2_sb[:nrow], op=AD)
                    ob = slice(rb * OUT, rb * OUT + OUT)
                    pb = slice(rb * PAD, rb * PAD + OUT)
                    # outp = dinv*h (Act, per-partition scale); D = 0.1*h; B = 0.1*outp
                    nc.scalar.activation(outp_sb[:nrow, ob], ht[:nrow], Copy,
                                         scale=dinv_sb[:nrow, rb:rb + 1])
                    nc.scalar.activation(D_sb[:nrow, ob], ht[:nrow], Copy,
                                         scale=ALPHA)
                    nc.scalar.activation(B_sb[:nrow, ob], outp_sb[:nrow, ob], Copy,
                                         scale=ALPHA)
                    nc.vector.tensor_copy(outp_pad[:nrow, pb], outp_sb[:nrow, ob])
                    nc.sync.dma_start(
                        out=outp_loc[r0:r0 + nrow, :],
                        in_=outp_pad[:nrow, rb * PAD:(rb + 1) * PAD])

            # ---------------- propagation ----------------
            range_lens = [min(RANGE, N - r * RANGE) for r in range(NRANGES)]
            for step in range(K):
                table = tables[step]
                nc.gpsimd.collective_compute(
                    "AllGather", mybir.AluOpType.bypass,
                    replica_groups=[list(range(NCORES))],
                    ins=[outp_loc[:].opt()],
                    outs=[table[:].opt()])
                with tc.tile_pool(name=f"gat{step}", bufs=6) as gat, \
                     tc.tile_pool(name=f"idx{step}", bufs=8) as idxp, \
                     tc.tile_pool(name=f"sbl{step}", bufs=6) as sbl, \
                     tc.tile_pool(name=f"ev{step}", bufs=4) as evp, \
                     tc.tile_pool(name=f"ps{step}", bufs=1, space="PSUM") as psp:
                    group_ps = {}
                    tile_cursor = 0
                    for ci, (r, ct) in enumerate(calls):
                        it = idxp.tile([128, CT * 8], mybir.dt.int16, tag="it")
                        nc.sync.dma_start(
                            out=it[:, :ct * 8],
                            in_=idxs_d[:, tile_cursor * 8:(tile_cursor + ct) * 8])
                        gt = gat.tile([128, CT, PAD], BF, tag="gt")
                        nc.gpsimd.dma_gather(
                            gt[:, :ct, :],
                            table[r * RANGE:r * RANGE + range_lens[r]],
                            it[:, :ct * 8],
                            ct * 128, ct * 128, PAD,
                            single_packet=False,
                            queue_num=ci % 4)
                        st = sbl.tile([128, CT * 128], BF, tag="st")
                        nc.vector.tensor_tensor(
                            out=st[:, :ct * 128].rearrange(
                                "p (a b) -> p a b", b=128),
                            in0=dstrel_bf[:, tile_cursor:tile_cursor + ct]
                                .unsqueeze(2).to_broadcast([128, ct, 128]),
                            in1=iota_bf[:, :ct * 128].rearrange(
                                "p (a b) -> p a b", b=128),
                            op=EQ)
                        done_groups = []
                        for j in range(ct):
                            gi, first, last = tiles[tile_cursor + j]
                            if first:
                                group_ps[gi] = psp.tile(
                                    [128, OUT], DT, tag=f"g{gi % 8}",
                                    name=f"ps{step}_{gi % 8}")
                            nc.tensor.matmul(
                                out=group_ps[gi][:, :],
                                lhsT=st[:, j * 128:(j + 1) * 128],
                                rhs=gt[:, j, 0:OUT],
                                start=first, stop=last)
                            if last:
                                done_groups.append(gi)
                        tile_cursor += ct
                        for gi in done_groups:
                            g = groups[gi]
                            b = g["b"]
                            nd = min(128, NLOC - b * 128)
                            ob = slice(b * OUT, b * OUT + OUT)
                            ps = group_ps.pop(gi)
                            if g["first_for_block"]:
                                nc.vector.tensor_copy(agg_sb[:, ob], ps[:, :])
                            else:
                                nc.vector.tensor_tensor(
                                    out=agg_sb[:, ob], in0=agg_sb[:, ob],
                                    in1=ps[:, :], op=AD)
                            if not g["last_for_block"]:
                                continue
                            # finalize block b
                            nc.vector.tensor_tensor(
                                out=agg_sb[:nd, ob], in0=agg_sb[:nd, ob],
                                in1=outp_sb[:nd, ob], op=AD)
                            if step < K - 1:
                                nc.scalar.activation(
                                    outp_sb[:nd, ob], agg_sb[:nd, ob], Copy,
                                    scale=A_sb[:nd, b:b + 1])
                                nc.vector.tensor_tensor(
                                    out=outp_sb[:nd, ob], in0=outp_sb[:nd, ob],
                                    in1=B_sb[:nd, ob], op=AD)
                                pb = slice(b * PAD, b * PAD + OUT)
                                nc.vector.tensor_copy(outp_pad[:nd, pb],
                                                      outp_sb[:nd, ob])
                                nc.sync.dma_start(
                                    out=outp_loc[b * 128:b * 128 + nd, :],
                                    in_=outp_pad[:nd, b * PAD:(b + 1) * PAD])
                            else:
                                res = evp.tile([128, OUT], DT, tag="res")
                                nc.scalar.activation(
                                    res[:nd], agg_sb[:nd, ob], Copy,
                                    scale=C_sb[:nd, b:b + 1])
                                nc.vector.tensor_tensor(
                                    out=res[:nd], in0=res[:nd],
                                    in1=D_sb[:nd, ob], op=AD)
                                nc.sync.dma_start(
                                    out=out_d[b * 128:b * 128 + nd, :],
                                    in_=res[:nd])
    nc.finalize()
    return nc


def kernel(x, edge_index, W1, b1, W2, b2):
    from concourse.bass_utils import run_bass_kernel_spmd

    in_maps, groups, tiles, calls, T_total = _preprocess(
        x, edge_index, W1, b1, W2, b2)
    nc = _build(groups, tiles, calls, T_total)
    trace = bool(int(os.environ.get("KERNEL_TRACE", "0")))
    if trace:
        import types
        mod = types.ModuleType("antenv.axon_hooks")
        mod._HOOK = None
        def _s(h): mod._HOOK = h
        def _g(): return mod._HOOK
        mod.set_axon_ntff_profile_hook = _s
        mod.get_axon_ntff_profile_hook = _g
        sys.modules["antenv.axon_hooks"] = mod
        import antenv
        antenv.axon_hooks = mod
        from trn_agent_boot.trn_boot import _ntff_profile_via_ctypes
        _s(_ntff_profile_via_ctypes('/opt/axon/libaxon_pjrt.so'))
        import concourse.bass_utils as bu
        bu.upload_artifacts = lambda tmpdir: "local://" + tmpdir
    res = run_bass_kernel_spmd(nc, in_maps, list(range(NCORES)), trace=trace)
    if trace and res.exec_time_ns:
        print(f"HW exec time: {res.exec_time_ns} ns")
    out = np.concatenate([res.results[c]["out"] for c in range(NCORES)], axis=0)
    return out
